# revision 44
# baseline (speedup 1.0000x reference)
"""Kalman CV filter (nn_KalmanCV) — Trainium2 Bass kernel, 8-core data parallel.

Math: the covariance P (and thus the Kalman gains K_t and the output
channels sx/sy/rho) is batch-independent — it depends only on the scalar
inputs. The whole per-batch computation therefore collapses to a linear
map over the 32 history scalars:

    out[l, b, ch<2] = sum_{t,ci} W[t*2+ci, l*2+ch] * hist[t, b, ci]
    out[l, b, ch>=2] = const[l, ch]          (sx, sy, rho)

Only the 50 data-dependent mu rows are computed on device; the 75
constant rows are filled host-side (they are input-data-independent,
like W itself). The matmul runs in fp16 (tolerance 2e-2, fp16 path
measures ~6e-4): 8x the fp32 PE rate and half the HBM bytes.

Layout: batch-stationary matmul packing PSUM to all 128 partitions.
Four batch quarter-shards are stacked on the partition axis: lhsT
(stationary) is a (128, 128) input slab whose rows 32j+c hold history
scalar c of batch quarter j; rhs (moving) is the block-diagonal weight
(128, 200) with W_mu (32, 50) in block j at [32j:32j+32, 50j:50j+50].
PSUM gets (128, 200) per matmul: out[m, 50j+p] = mu_p of quarter-j
batch element m — every PSUM partition carries real output, so the PE
streams 200 columns per 512 batch elements (the dense minimum).
PSUM->SBUF f16 cast-copies alternate the Scalar/Vector engines, and
output DMA kicks alternate the gpsimd/sync rings.
"""
import numpy as np

DT = 0.2
LEN_HIST = 16
LEN_PRED = 25
BATCH = 100000

N_CORES = 8
BS_REAL = BATCH // N_CORES  # 12500 batch per core
QB_REAL = BS_REAL // 4      # 3125 per quarter-shard
QB = 3200                   # padded quarter (25 matmul slabs of 128)
NMM = QB // 128             # 25 matmuls per core
NW = 200                    # moving columns per matmul (4 quarters x 50)
OUTW = NMM * NW             # 5000 output columns per core

# The first DMA carries w fused with the first FUSED_SLABS input slabs:
# one ring completion unblocks the first three matmuls. Later input
# chunks are sized to ~6 matmuls each (ring completions arrive roughly
# one per microsecond).
FUSED_SLABS = 7
WXW = NW + FUSED_SLABS * 128            # 1096 columns in the fused tile
IN_CHUNKS = [(896, 768), (1664, 768), (2432, 768)]
# Output chunks over the 5000 out columns: 3 PSUM tiles (6 matmuls) per
# chunk; tiny tail chunk keeps the final DMA drain short.
OUT_CHUNKS = [(0, 1200), (1200, 1200), (2400, 1200), (3600, 1200),
              (4800, 200)]
PSUM_MM = 2                 # matmuls per PSUM tile (1600B x f32 = 1 bank)


def _build_wc(vsx, vsy, asx, asy, GR, coef_G, len_pred):
    """Collapse the filter to W (32, 5L) and constant vector cvec (5L,)."""
    L = int(len_pred)
    H = np.zeros((2, 4)); H[0, 0] = 1.0; H[1, 2] = 1.0
    F = np.eye(4); F[0, 1] = DT; F[2, 3] = DT
    G = np.array([DT * DT / 2, DT, DT * DT / 2, DT])
    Id = np.eye(4)

    ax2 = float(asx[0]) ** 2
    ay2 = float(asy[0]) ** 2
    mx = np.array([1.0, 1.0, 0.0, 0.0]); my = 1.0 - mx
    scale = (ax2 * np.outer(mx, mx) + ay2 * np.outer(my, my)
             + np.outer(mx, my) + np.outer(my, mx))
    g = G * np.tanh(np.asarray(coef_G, np.float64))
    Q = np.outer(g, g) * scale
    R = np.outer(np.asarray(GR, np.float64), np.asarray(GR, np.float64))

    D0 = np.array([[1.0, 0.0], [-1.0 / DT, 0.0], [0.0, 1.0], [0.0, -1.0 / DT]])
    D1 = np.array([[0.0, 0.0], [1.0 / DT, 0.0], [0.0, 0.0], [0.0, 1.0 / DT]])
    P = np.diag([R[0, 0], float(vsx[0]) ** 2, R[1, 1], float(vsy[0]) ** 2])

    C = np.zeros((LEN_HIST, 4, 2))
    C[0] = D0; C[1] = D1
    for t in range(1, LEN_HIST):
        P = F @ P @ F.T + Q
        S = H @ P @ H.T + R
        K = P @ H.T @ np.linalg.inv(S)
        A = (Id - K @ H) @ F
        C = np.einsum('ij,tjk->tik', A, C)
        C[t] += K
        ImKH = Id - K @ H
        P = ImKH @ P @ ImKH.T + K @ R @ K.T

    W_dev = np.zeros((2 * LEN_HIST, 5 * L))
    cvec = np.zeros(5 * L)
    M = np.eye(4)
    for l in range(L):
        M = F @ M
        P = F @ P @ F.T + Q
        HFl = H @ M
        Wl = np.einsum('ij,tjk->itk', HFl, C)   # (2, T, 2)
        for ch in range(2):
            W_dev[:, l * 5 + ch] = Wl[ch].reshape(-1)
        Pout = H @ P @ H.T
        sx = np.sqrt(Pout[0, 0]); sy = np.sqrt(Pout[1, 1])
        cvec[l * 5 + 2] = sx
        cvec[l * 5 + 3] = sy
        cvec[l * 5 + 4] = (Pout[0, 1] + Pout[1, 0]) / (2.0 * sx * sy)
    return W_dev.astype(np.float32), cvec.astype(np.float32)


_NC_CACHE = {}


def _build_bass():
    import concourse.bass as bass
    import concourse.bacc as bacc
    import concourse.tile as tile
    from concourse import mybir

    nc = bacc.Bacc("TRN2", target_bir_lowering=False, debug=False,
                   num_devices=N_CORES)
    wx = nc.declare_dram_parameter("wx", [128, WXW], mybir.dt.float16,
                                   isOutput=False)
    x = nc.declare_dram_parameter("x", [128, QB - FUSED_SLABS * 128],
                                  mybir.dt.float16, isOutput=False)
    out = nc.declare_dram_parameter("out", [128, OUTW], mybir.dt.float16,
                                    isOutput=True)

    with tile.TileContext(nc) as tc:
        with tc.tile_pool(name="singles", bufs=1) as singles, \
             tc.tile_pool(name="xin", bufs=4) as xin_pool, \
             tc.tile_pool(name="ps", bufs=6, space="PSUM") as psum_pool, \
             tc.tile_pool(name="op", bufs=5) as out_pool:
            # Sync exits the framework preamble earliest and its ring
            # has the lowest completion latency: the fused w+first-slabs
            # tile goes first, then the remaining x chunks.
            wx_tile = singles.tile([128, WXW], mybir.dt.float16)
            nc.sync.dma_start(out=wx_tile, in_=wx[:, :])
            w_tile = wx_tile[:, 0:NW]
            x_tiles = []
            for ioff, iw in IN_CHUNKS:
                t = xin_pool.tile([128, iw], mybir.dt.float16)
                nc.sync.dma_start(
                    out=t, in_=x[:, ioff - FUSED_SLABS * 128:
                                 ioff - FUSED_SLABS * 128 + iw])
                x_tiles.append((ioff, iw, t))

            def x_slab(i):
                off = i * 128
                if i < FUSED_SLABS:
                    return wx_tile[:, NW + off:NW + off + 128]
                for ioff, iw, t in x_tiles:
                    if ioff <= off and off + 128 <= ioff + iw:
                        return t[:, off - ioff:off - ioff + 128]
                raise AssertionError("no input tile covers request")

            mm = 0
            tidx = 0
            for c, (goff, gw) in enumerate(OUT_CHUNKS):
                o_tile = out_pool.tile([128, gw], mybir.dt.float16)
                off = 0
                while off < gw:
                    nmm = min(PSUM_MM, (gw - off) // NW)
                    pw = nmm * NW
                    ps = psum_pool.tile([128, pw], mybir.dt.float32)
                    for k in range(nmm):
                        nc.tensor.matmul(ps[:, k * NW:(k + 1) * NW],
                                         x_slab(mm), w_tile,
                                         start=True, stop=True)
                        mm += 1
                    dst = o_tile[:, off:off + pw]
                    # Alternate copy engines per PSUM tile so both halves
                    # of a chunk are cast concurrently.
                    if tidx % 2 == 0:
                        nc.scalar.copy(out=dst, in_=ps)
                    else:
                        nc.vector.tensor_copy(out=dst, in_=ps)
                    tidx += 1
                    off += pw
                # Alternate output kicks across the gpsimd and sync
                # rings; the tiny tail chunk lands on gpsimd right after
                # its previous transfer so the final drain is short.
                eng = nc.gpsimd if c % 2 == 0 else nc.sync
                eng.dma_start(out=out[:, goff:goff + gw], in_=o_tile)
    nc.compile()
    return nc


def _get_nc():
    if "nc" not in _NC_CACHE:
        _NC_CACHE["nc"] = _build_bass()
    return _NC_CACHE["nc"]


def _run_device(x_shards, W4, trace=False):
    from concourse.bass_utils import run_bass_kernel_spmd

    split = FUSED_SLABS * 128
    in_maps = [
        {"wx": np.ascontiguousarray(
            np.concatenate([W4, shard[:, :split]], axis=1)),
         "x": np.ascontiguousarray(shard[:, split:])}
        for shard in x_shards
    ]
    return run_bass_kernel_spmd(_get_nc(), in_maps, list(range(N_CORES)),
                                trace=trace)


def _make_shards(hist_T16):
    """hist_T16: (32, BATCH) f16 -> per-core (128, QB) quarter-stacked."""
    shards = []
    for c in range(N_CORES):
        xc = hist_T16[:, c * BS_REAL:(c + 1) * BS_REAL]  # (32, 12500)
        shard = np.zeros((128, QB), np.float16)
        for j in range(4):
            shard[32 * j:32 * j + 32, :QB_REAL] = \
                xc[:, j * QB_REAL:(j + 1) * QB_REAL]
        shards.append(shard)
    return shards


def _make_w4(W_mu16):
    """W_mu16 (32, 50) f16 -> block-diagonal (128, NW)."""
    W4 = np.zeros((128, NW), np.float16)
    for j in range(4):
        W4[32 * j:32 * j + 32, 50 * j:50 * j + 50] = W_mu16
    return W4


def kernel(hist, velocity_std_x, velocity_std_y, acceleration_std_x,
           acceleration_std_y, GR, coef_G, len_pred):
    hist = np.asarray(hist, np.float32)
    L = int(len_pred)
    W, cvec = _build_wc(velocity_std_x, velocity_std_y, acceleration_std_x,
                        acceleration_std_y, GR, coef_G, L)
    T, B, _ = hist.shape
    hist_T = np.ascontiguousarray(hist.transpose(0, 2, 1)).reshape(2 * T, B)

    if L != LEN_PRED or B != BATCH or T != LEN_HIST:
        # shape surprise: fall back to exact host math
        out_flat = W.T @ hist_T + cvec[:, None]
        return np.ascontiguousarray(
            out_flat.reshape(L, 5, B).transpose(0, 2, 1)).astype(np.float32)

    mu_cols = np.array([l * 5 + ch for l in range(LEN_PRED) for ch in range(2)])
    W_mu = W[:, mu_cols].astype(np.float16)   # (32, 50)

    res = _run_device(_make_shards(hist_T.astype(np.float16)), _make_w4(W_mu))

    out = np.empty((LEN_PRED, B, 5), np.float32)
    consts = cvec.reshape(LEN_PRED, 5)[:, 2:5]           # (25, 3)
    out[:, :, 2:5] = consts[:, None, :]
    for c in range(N_CORES):
        oc = res.results[c]["out"]                       # (128, 5000) f16
        # oc[m, NW*i + 50*j + p] = mu_p of batch quarter j, element 128i+m
        arr = oc.reshape(128, NMM, 4, 50).transpose(3, 2, 1, 0)  # (50,4,25,128)
        mu = arr.reshape(50, 4, QB)[:, :, :QB_REAL].reshape(50, BS_REAL)
        out[:, c * BS_REAL:(c + 1) * BS_REAL, 0:2] = (
            mu.reshape(LEN_PRED, 2, BS_REAL).transpose(0, 2, 1))
    return out
